# revision 1
# baseline (speedup 1.0000x reference)
"""Dense MoE forward for Trainium2: 8-core data-parallel SPMD Bass/Tile kernel.

Reference computation (per token row x[b, :], all experts dense):
    gates = softmax(x @ Wg + bg)                      # [B, E]
    h_e   = relu(x @ W1[e] + b1[e])                   # [B, H] per expert
    eo_e  = h_e @ W2[e] + b2[e]                       # [B, C]
    out   = sum_e gates[:, e] * eo_e                  # [B, C]

Strategy (per core, B_loc = B/8 tokens):
  - x is pre-cast to fp16 on host; DMA-transposed on load so D sits on SBUF
    partitions (xT tiles [128d x 512tok]).
  - Layer 1 feature-major: hT[e] = W1[e].T @ xT (PE, fp16 in / fp32 psum),
    relu+bias on ACT (per-partition bias), output fp16.
  - Gate logits token-major [tok, E]; softmax over free dim (exp on ACT,
    sum+reciprocal+normalize on DVE). Gate bias applied multiplicatively:
    u = exp(logits) * exp(bg), with exp(bg) host-replicated across partitions.
  - Layer 2 token-major: eo[t*128:, :] = hT-slice.T @ W2[e], accumulated in
    PSUM. No bias matmuls: b2 is host-replicated across partitions and enters
    through the combine (small-N K=1 matmuls measured ~as expensive as real
    work, so they were eliminated).
  - Expert combine on DVE, per (expert, subtile): two fused ops
    acc = (eo_e * g_col_e) + acc; acc = (b2_e * g_col_e) + acc
    (scalar_tensor_tensor; the gate is a per-partition [128,1] scalar in
    token-major layout).

fp16 products are exact in fp32 accumulation, so matmul error is only the
fp16 input rounding (~2.4e-4 relative).
"""

import numpy as np

from concourse import bacc, bass, mybir, tile
from concourse.bass_utils import run_bass_kernel_spmd

B, D, C, E, H = 65536, 512, 101, 8, 256
N_CORES = 8
BL = B // N_CORES          # 8192 tokens per core
MACRO = 512                # tokens per macro tile (matmul moving-dim max)
SUB = 128                  # tokens per subtile (PE stationary free-dim max)
NSUB = MACRO // SUB        # 4
DK = D // 128              # 4 contraction chunks for layer 1 / gate
HJ = H // 128              # 2 contraction chunks for layer 2

F16 = mybir.dt.float16
F32 = mybir.dt.float32
AF = mybir.ActivationFunctionType
ALU = mybir.AluOpType


def build_nc(n_tokens: int = BL, repeat: int = 1) -> bass.Bass:
    """repeat > 1 re-runs the whole body in a hardware loop (same data,
    same outputs) — used only for timing measurements via wall-clock slope."""
    assert n_tokens % MACRO == 0
    n_macro = n_tokens // MACRO

    nc = bacc.Bacc("TRN2", debug=False)

    x = nc.dram_tensor("x", [n_tokens, D], F16, kind="ExternalInput").ap()
    w1 = nc.dram_tensor("w1", [128, DK, E, H], F16, kind="ExternalInput").ap()
    w2 = nc.dram_tensor("w2", [128, HJ, E, C], F16, kind="ExternalInput").ap()
    wg = nc.dram_tensor("wg", [128, DK, E], F16, kind="ExternalInput").ap()
    b1 = nc.dram_tensor("b1", [128, HJ, E], F32, kind="ExternalInput").ap()
    # b2 replicated across partitions (token-major bias add on DVE),
    # exp(bg) replicated across partitions and subtiles (gate reweighting).
    b2 = nc.dram_tensor("b2", [128, E, C], F32, kind="ExternalInput").ap()
    bg = nc.dram_tensor("bg", [128, NSUB, E], F32, kind="ExternalInput").ap()
    out = nc.dram_tensor("out", [n_tokens, C], F32, kind="ExternalOutput").ap()

    with tile.TileContext(nc) as tc:
        with (
            tc.tile_pool(name="wpool", bufs=1) as wpool,
            tc.tile_pool(name="xpool", bufs=2) as xpool,
            tc.tile_pool(name="hpool", bufs=2) as hpool,
            tc.tile_pool(name="gpool", bufs=2) as gpool,
            tc.tile_pool(name="apool", bufs=2) as apool,
            tc.tile_pool(name="pgp", bufs=1, space="PSUM") as pgp,
            tc.tile_pool(name="php", bufs=3, space="PSUM") as php,
            tc.tile_pool(name="peop", bufs=4, space="PSUM") as peop,
        ):
            # ---- persistent weights ----
            w1s = wpool.tile([128, DK, E, H], F16)
            w2s = wpool.tile([128, HJ, E, C], F16)
            wgs = wpool.tile([128, DK, E], F16)
            b1s = wpool.tile([128, HJ, E], F32)
            b2s = wpool.tile([128, E, C], F32)
            bgs = wpool.tile([128, NSUB, E], F32)

            nc.sync.dma_start(out=w1s[:], in_=w1)
            nc.sync.dma_start(out=w2s[:], in_=w2)
            nc.sync.dma_start(out=wgs[:], in_=wg)
            nc.sync.dma_start(out=b1s[:], in_=b1)
            nc.sync.dma_start(out=b2s[:], in_=b2)
            nc.sync.dma_start(out=bgs[:], in_=bg)

            import contextlib
            rep_ctx = (
                tc.For_i(0, repeat, 1) if repeat > 1
                else contextlib.nullcontext()
            )
            with rep_ctx:
                body(nc, tc, n_macro, x, out, w1s, w2s, wgs, b1s, b2s, bgs,
                     xpool, hpool, gpool, apool, pgp, php, peop)

    nc.compile()
    return nc


def body(nc, tc, n_macro, x, out, w1s, w2s, wgs, b1s, b2s, bgs,
         xpool, hpool, gpool, apool, pgp, php, peop):
    if True:
            for m in range(n_macro):
                t0 = m * MACRO

                # ---- xT: [128 d, MACRO tok] per d-chunk via DMA transpose ----
                xt = xpool.tile([128, DK, MACRO], F16, tag="xt")
                for k in range(DK):
                    nc.sync.dma_start_transpose(
                        xt[:, k], x[t0:t0 + MACRO, k * 128:(k + 1) * 128]
                    )

                # ---- gates (token-major logits, softmax over E) ----
                # u = exp(x @ Wg) * exp(bg); normalize by its row sum.
                pg = pgp.tile([128, NSUB, E], F32, tag="pg")
                for t in range(NSUB):
                    for k in range(DK):
                        nc.tensor.matmul(
                            pg[:, t],
                            lhsT=xt[:, k, t * SUB:(t + 1) * SUB],
                            rhs=wgs[:, k],
                            start=(k == 0),
                            stop=(k == DK - 1),
                        )
                u = gpool.tile([128, NSUB, E], F32, tag="u")
                nc.scalar.activation(u[:], pg[:], AF.Exp)
                u2 = gpool.tile([128, NSUB, E], F32, tag="u2")
                nc.vector.tensor_mul(u2[:], u[:], bgs[:])
                s = gpool.tile([128, NSUB], F32, tag="s")
                nc.vector.reduce_sum(s[:], u2[:], axis=mybir.AxisListType.X)
                r = gpool.tile([128, NSUB], F32, tag="r")
                nc.vector.reciprocal(r[:], s[:])
                g = gpool.tile([128, NSUB, E], F32, tag="g")
                for t in range(NSUB):
                    nc.vector.tensor_scalar_mul(g[:, t], u2[:, t], r[:, t:t + 1])

                acc = apool.tile([128, NSUB, C], F32, tag="acc")

                def emit_l2(e, ht):
                    """Layer 2 + gated combine for expert e (token-major).
                    acc += g_e * (h_e @ W2_e) + g_e * b2_e; the b2 term uses
                    the partition-replicated b2s tile on DVE (no PE matmul).
                    """
                    for t in range(NSUB):
                        peo = peop.tile([128, C], F32, tag="peo", name="peo")
                        for j in range(HJ):
                            nc.tensor.matmul(
                                peo[:],
                                lhsT=ht[:, j, t * SUB:(t + 1) * SUB],
                                rhs=w2s[:, j, e],
                                start=(j == 0),
                                stop=(j == HJ - 1),
                            )
                        if e == 0:
                            nc.vector.tensor_scalar_mul(
                                acc[:, t], peo[:], g[:, t, e:e + 1]
                            )
                        else:
                            nc.vector.scalar_tensor_tensor(
                                acc[:, t], peo[:], g[:, t, e:e + 1], acc[:, t],
                                op0=ALU.mult, op1=ALU.add,
                            )
                        nc.vector.scalar_tensor_tensor(
                            acc[:, t], b2s[:, e], g[:, t, e:e + 1], acc[:, t],
                            op0=ALU.mult, op1=ALU.add,
                        )

                # ---- experts: L1(e) emitted before L2(e-1) so the PE always
                # has independent matmul work while ACT runs relu(e). ----
                pending = None
                for e in range(E):
                    phs = [
                        php.tile([128, MACRO], F32, tag="ph", name="ph")
                        for _ in range(HJ)
                    ]
                    for j in range(HJ):
                        for k in range(DK):
                            nc.tensor.matmul(
                                phs[j][:],
                                lhsT=w1s[:, k, e, j * 128:(j + 1) * 128],
                                rhs=xt[:, k],
                                start=(k == 0),
                                stop=(k == DK - 1),
                            )
                    ht = hpool.tile([128, HJ, MACRO], F16, tag="ht", name="ht")
                    for j in range(HJ):
                        nc.scalar.activation(
                            ht[:, j], phs[j][:], AF.Relu, bias=b1s[:, j, e:e + 1]
                        )
                    if pending is not None:
                        emit_l2(*pending)
                    pending = (e, ht)
                emit_l2(*pending)

                # ---- store ----
                for t in range(NSUB):
                    nc.sync.dma_start(
                        out=out[t0 + t * SUB:t0 + (t + 1) * SUB, :],
                        in_=acc[:, t],
                    )


def _prep_weights(W1, b1, W2, b2, Wg, bg):
    w1p = np.ascontiguousarray(
        W1.astype(np.float16).transpose(1, 0, 2).reshape(DK, 128, E, H)
        .transpose(1, 0, 2, 3)
    )
    w2p = np.ascontiguousarray(
        W2.astype(np.float16).transpose(1, 0, 2).reshape(HJ, 128, E, C)
        .transpose(1, 0, 2, 3)
    )
    wgp = np.ascontiguousarray(
        Wg.astype(np.float16).reshape(DK, 128, E).transpose(1, 0, 2)
    )
    b1p = np.ascontiguousarray(
        b1.astype(np.float32).T.reshape(HJ, 128, E).transpose(1, 0, 2)
    )
    b2p = np.ascontiguousarray(
        np.broadcast_to(b2.astype(np.float32), (128, E, C))
    )
    bgp = np.ascontiguousarray(np.broadcast_to(
        np.exp(bg).astype(np.float32), (128, NSUB, E)
    ))
    return w1p, w2p, wgp, b1p, b2p, bgp


_CACHE: dict = {}


def kernel(x, W1, b1, W2, b2, Wg, bg, _trace=False):
    x = np.asarray(x, dtype=np.float32)
    W1 = np.asarray(W1, dtype=np.float32)
    b1 = np.asarray(b1, dtype=np.float32)
    W2 = np.asarray(W2, dtype=np.float32)
    b2 = np.asarray(b2, dtype=np.float32)
    Wg = np.asarray(Wg, dtype=np.float32)
    bg = np.asarray(bg, dtype=np.float32)

    if "nc" not in _CACHE:
        _CACHE["nc"] = build_nc()
    nc = _CACHE["nc"]

    x16 = x.astype(np.float16)
    w1p, w2p, wgp, b1p, b2p, bgp = _prep_weights(W1, b1, W2, b2, Wg, bg)

    in_maps = [
        {
            "x": x16[i * BL:(i + 1) * BL],
            "w1": w1p, "w2": w2p, "wg": wgp,
            "b1": b1p, "b2": b2p, "bg": bgp,
        }
        for i in range(N_CORES)
    ]
    try:
        res = run_bass_kernel_spmd(
            nc, in_maps, core_ids=list(range(N_CORES)), trace=_trace
        )
    except ModuleNotFoundError:
        # NTFF profile hook unavailable in this container — run untraced.
        res = run_bass_kernel_spmd(
            nc, in_maps, core_ids=list(range(N_CORES)), trace=False
        )
    out = np.concatenate(
        [res.results[i]["out"] for i in range(N_CORES)], axis=0
    )
    if _trace:
        _CACHE["last_result"] = res
    return out



# revision 2
# speedup vs baseline: 1.2195x; 1.2195x over previous
"""Dense MoE forward for Trainium2: 8-core data-parallel SPMD Bass/Tile kernel.

Reference computation (per token row x[b, :], all experts dense):
    gates = softmax(x @ Wg + bg)                      # [B, E]
    h_e   = relu(x @ W1[e] + b1[e])                   # [B, H] per expert
    eo_e  = h_e @ W2[e] + b2[e]                       # [B, C]
    out   = sum_e gates[:, e] * eo_e                  # [B, C]

Strategy (per core, B_loc = B/8 = 8192 tokens, data-parallel over cores):
  - x is cast to fp16 and pre-transposed on host to [128 d-part, macro,
    k-chunk, token] so the device does plain (non-transposing) DMA loads.
  - Tokens are processed in 1024-token super-macros (2 x 512-token halves).
    Layer 1 is feature-major: hT[e] = W1[e].T @ xT with fp32 PSUM
    accumulation over the 4 k-chunks of D=512.
  - Each W1 stationary [128d x 128h] is loaded once and feeds BOTH
    512-token halves (two matmuls into two PSUM banks). The tile
    legalizer emits one LDWEIGHTS per matmul unconditionally; hardware
    keeps the stationary across matmuls, so dedupe_ldweights() removes
    back-to-back duplicate InstLdweights from the final scheduled BIR
    (measured ~100ns each on HW). This is safe by construction: a
    duplicate is only removed when it is immediately adjacent to an
    identical load in the final PE program order and carries no
    semaphore waits/updates.
  - Gate logits token-major [tok, E]; softmax over the free dim (exp on
    ACT with multiplicative exp(bg), sum/reciprocal/normalize on DVE).
  - Layer 2 token-major: eo[t*128:, :] = hT-slice.T @ W2[e] accumulated
    in PSUM over the two h-chunks; gate-weighted combine and the b2 term
    on DVE via scalar_tensor_tensor with the per-partition gate column.
  - One merged output DMA per super-macro through a [B/128, 128, C] view
    of the output tensor.
"""

import contextlib

import numpy as np

from concourse import bacc, bass, mybir, tile
from concourse.bass_utils import run_bass_kernel_spmd

B, D, C, E, H = 65536, 512, 101, 8, 256
N_CORES = 8
BL = B // N_CORES     # 8192 tokens per core
MACRO = 512           # tokens per sub-macro (matmul moving dim)
NSUB = 4              # 128-token subtiles per sub-macro
SUB = 128
SMAC = 2              # sub-macros per super-macro (W1 stationary reuse)
SUPER = SMAC * MACRO  # 1024
DK = D // 128
HJ = H // 128

F16 = mybir.dt.float16
F32 = mybir.dt.float32
AF = mybir.ActivationFunctionType
ALU = mybir.AluOpType


def dedupe_ldweights(nc):
    """Remove back-to-back duplicate InstLdweights from the compiled BIR.

    Only an InstLdweights whose lowered access pattern is identical to the
    immediately preceding InstLdweights on the PE stream (no other weight
    load or transpose between), and which carries no semaphore waits or
    updates, is dropped. Matmuls between the two loads do not touch the
    PE weight registers, so the second load is redundant by definition.
    """
    removed = 0
    for blk in nc.m.functions[0].blocks:
        insts = blk.instructions
        new = []
        last_key = None
        changed = False
        for i in insts:
            tn = type(i).__name__
            if tn == "InstLdweights":
                key = (repr(i.ins[0]), str(i.perf_mode), str(i.tile_position),
                       str(i.is_transpose))
                if (key == last_key and not i.has_wait()
                        and not i.has_update()):
                    removed += 1
                    changed = True
                    continue
                last_key = key
            elif tn in ("InstMatmult", "InstMatmultMx"):
                if getattr(i, "is_transpose", None):
                    last_key = None
            new.append(i)
        if changed:
            blk.instructions = new
    return removed


def build_nc(n_tokens: int = BL, repeat: int = 1) -> bass.Bass:
    """repeat > 1 re-runs the whole body in a hardware loop (same data,
    same outputs) — used only for timing measurements via wall-clock slope."""
    assert n_tokens % SUPER == 0
    n_super = n_tokens // SUPER
    nm512 = n_tokens // MACRO

    nc = bacc.Bacc("TRN2", debug=False)

    xd = nc.dram_tensor("x", [128, nm512, DK * MACRO], F16,
                        kind="ExternalInput").ap()
    w1 = nc.dram_tensor("w1", [128, DK, E, H], F16, kind="ExternalInput").ap()
    w2 = nc.dram_tensor("w2", [128, HJ, E, C], F16, kind="ExternalInput").ap()
    wg = nc.dram_tensor("wg", [128, DK, E], F16, kind="ExternalInput").ap()
    b1 = nc.dram_tensor("b1", [128, HJ, E], F32, kind="ExternalInput").ap()
    b2 = nc.dram_tensor("b2", [128, E, C], F32, kind="ExternalInput").ap()
    bg = nc.dram_tensor("bg", [128, SMAC, NSUB, E], F32,
                        kind="ExternalInput").ap()
    # out viewed [a, 128, C]: token a*128+p — same bytes as [n_tokens, C]
    out = nc.dram_tensor("out", [n_tokens // SUB, SUB, C], F32,
                         kind="ExternalOutput").ap()

    with tile.TileContext(nc) as tc:
        with (
            tc.tile_pool(name="wpool", bufs=1) as wpool,
            tc.tile_pool(name="xpool", bufs=2) as xpool,
            tc.tile_pool(name="hpool", bufs=2) as hpool,
            tc.tile_pool(name="gpool", bufs=2) as gpool,
            tc.tile_pool(name="apool", bufs=2) as apool,
            tc.tile_pool(name="pgp", bufs=1, space="PSUM") as pgp,
            tc.tile_pool(name="php", bufs=4, space="PSUM") as php,
            tc.tile_pool(name="peop", bufs=3, space="PSUM") as peop,
        ):
            w1s = wpool.tile([128, DK, E, H], F16)
            w2s = wpool.tile([128, HJ, E, C], F16)
            wgs = wpool.tile([128, DK, E], F16)
            b1s = wpool.tile([128, HJ, E], F32)
            b2s = wpool.tile([128, E, C], F32)
            bgs = wpool.tile([128, SMAC, NSUB, E], F32)

            nc.sync.dma_start(out=w1s[:], in_=w1)
            nc.sync.dma_start(out=w2s[:], in_=w2)
            nc.sync.dma_start(out=wgs[:], in_=wg)
            nc.sync.dma_start(out=b1s[:], in_=b1)
            nc.sync.dma_start(out=b2s[:], in_=b2)
            nc.sync.dma_start(out=bgs[:], in_=bg)

            rep_ctx = (
                tc.For_i(0, repeat, 1) if repeat > 1
                else contextlib.nullcontext()
            )
            with rep_ctx:
                for M in range(n_super):
                    emit_super(nc, M, xd, out, w1s, w2s, wgs, b1s, b2s, bgs,
                               xpool, hpool, gpool, apool, pgp, php, peop)

    nc.compile()
    dedupe_ldweights(nc)
    return nc


def emit_super(nc, M, xd, out, w1s, w2s, wgs, b1s, b2s, bgs,
               xpool, hpool, gpool, apool, pgp, php, peop):
    xt = xpool.tile([128, SMAC, DK, MACRO], F16, tag="xt")
    nc.sync.dma_start(out=xt[:], in_=xd[:, SMAC * M:SMAC * (M + 1)])

    # ---- gates (token-major logits, softmax over E) ----
    # u = exp(x @ Wg) * exp(bg); normalize by its row sum on DVE.
    pg = pgp.tile([128, SMAC, NSUB, E], F32, tag="pg")
    for s in range(SMAC):
        for t in range(NSUB):
            for k in range(DK):
                nc.tensor.matmul(
                    pg[:, s, t],
                    lhsT=xt[:, s, k, t * SUB:(t + 1) * SUB],
                    rhs=wgs[:, k],
                    start=(k == 0),
                    stop=(k == DK - 1),
                )
    u = gpool.tile([128, SMAC, NSUB, E], F32, tag="u")
    nc.scalar.activation(u[:], pg[:], AF.Exp)
    u2 = gpool.tile([128, SMAC, NSUB, E], F32, tag="u2")
    nc.vector.tensor_mul(u2[:], u[:], bgs[:])
    sm = gpool.tile([128, SMAC, NSUB], F32, tag="s")
    nc.vector.reduce_sum(sm[:], u2[:], axis=mybir.AxisListType.X)
    r = gpool.tile([128, SMAC, NSUB], F32, tag="r")
    nc.vector.reciprocal(r[:], sm[:])
    g = gpool.tile([128, SMAC, NSUB, E], F32, tag="g")
    for s in range(SMAC):
        for t in range(NSUB):
            nc.vector.tensor_scalar_mul(g[:, s, t], u2[:, s, t],
                                        r[:, s, t:t + 1])

    acc = apool.tile([128, SMAC * NSUB, C], F32, tag="acc")

    def emit_l2(e, ht):
        """Layer 2 + gated combine for expert e (token-major).
        acc += g_e * (h_e @ W2_e) + g_e * b2_e; the b2 term uses the
        partition-replicated b2s tile on DVE (no PE matmul)."""
        for t in range(SMAC * NSUB):
            peo = peop.tile([128, C], F32, tag="peo", name="peo")
            for j in range(HJ):
                nc.tensor.matmul(
                    peo[:],
                    lhsT=ht[:, j, t * SUB:(t + 1) * SUB],
                    rhs=w2s[:, j, e],
                    start=(j == 0),
                    stop=(j == HJ - 1),
                )
            gcol = g[:, t // NSUB, t % NSUB, e:e + 1]
            if e == 0:
                nc.vector.tensor_scalar_mul(acc[:, t], peo[:], gcol)
            else:
                nc.vector.scalar_tensor_tensor(
                    acc[:, t], peo[:], gcol, acc[:, t],
                    op0=ALU.mult, op1=ALU.add,
                )
            nc.vector.scalar_tensor_tensor(
                acc[:, t], b2s[:, e], gcol, acc[:, t],
                op0=ALU.mult, op1=ALU.add,
            )

    # ---- experts: each W1 stationary feeds both 512-token halves; L1(e)
    # is emitted before L2(e-1) so the PE always has independent matmul
    # work while ACT runs relu(e). ----
    pending = None
    for e in range(E):
        ht = hpool.tile([128, HJ, SUPER], F16, tag="ht", name="ht")
        for j in range(HJ):
            phs = [
                php.tile([128, MACRO], F32, tag="ph", name="ph")
                for _ in range(SMAC)
            ]
            for k in range(DK):
                for h in range(SMAC):
                    nc.tensor.matmul(
                        phs[h][:],
                        lhsT=w1s[:, k, e, j * 128:(j + 1) * 128],
                        rhs=xt[:, h, k],
                        start=(k == 0),
                        stop=(k == DK - 1),
                    )
            for h in range(SMAC):
                nc.scalar.activation(
                    ht[:, j, h * MACRO:(h + 1) * MACRO], phs[h][:],
                    AF.Relu, bias=b1s[:, j, e:e + 1]
                )
        if pending is not None:
            emit_l2(*pending)
        pending = (e, ht)
    emit_l2(*pending)

    # ---- store: one DMA for the whole super-macro ----
    a0 = M * (SUPER // SUB)
    nc.sync.dma_start(
        out=out[a0:a0 + SMAC * NSUB].rearrange("a p c -> p a c"),
        in_=acc[:],
    )


def _prep_weights(W1, b1, W2, b2, Wg, bg):
    w1p = np.ascontiguousarray(
        W1.astype(np.float16).transpose(1, 0, 2).reshape(DK, 128, E, H)
        .transpose(1, 0, 2, 3)
    )
    w2p = np.ascontiguousarray(
        W2.astype(np.float16).transpose(1, 0, 2).reshape(HJ, 128, E, C)
        .transpose(1, 0, 2, 3)
    )
    wgp = np.ascontiguousarray(
        Wg.astype(np.float16).reshape(DK, 128, E).transpose(1, 0, 2)
    )
    b1p = np.ascontiguousarray(
        b1.astype(np.float32).T.reshape(HJ, 128, E).transpose(1, 0, 2)
    )
    b2p = np.ascontiguousarray(
        np.broadcast_to(b2.astype(np.float32), (128, E, C))
    )
    bgp = np.ascontiguousarray(np.broadcast_to(
        np.exp(bg).astype(np.float32), (128, SMAC, NSUB, E)
    ))
    return w1p, w2p, wgp, b1p, b2p, bgp


def prep_x(x16_core):
    """[BL, 512] fp16 -> [128, n_macro512, DK*512], host-transposed so
    element [p, m, k*512+t] = x[m*512+t, k*128+p]."""
    nm = x16_core.shape[0] // MACRO
    return np.ascontiguousarray(
        x16_core.reshape(nm, MACRO, DK, 128).transpose(3, 0, 2, 1)
        .reshape(128, nm, DK * MACRO)
    )


def make_in_maps(inputs):
    x16 = np.asarray(inputs["x"], np.float32).astype(np.float16)
    w1p, w2p, wgp, b1p, b2p, bgp = _prep_weights(
        np.asarray(inputs["W1"], np.float32),
        np.asarray(inputs["b1"], np.float32),
        np.asarray(inputs["W2"], np.float32),
        np.asarray(inputs["b2"], np.float32),
        np.asarray(inputs["Wg"], np.float32),
        np.asarray(inputs["bg"], np.float32))
    return [
        {
            "x": prep_x(x16[i * BL:(i + 1) * BL]),
            "w1": w1p, "w2": w2p, "wg": wgp,
            "b1": b1p, "b2": b2p, "bg": bgp,
        }
        for i in range(N_CORES)
    ]


_CACHE: dict = {}


def kernel(x, W1, b1, W2, b2, Wg, bg):
    if "nc" not in _CACHE:
        _CACHE["nc"] = build_nc()
    nc = _CACHE["nc"]

    in_maps = make_in_maps({
        "x": x, "W1": W1, "b1": b1, "W2": W2, "b2": b2, "Wg": Wg, "bg": bg,
    })
    res = run_bass_kernel_spmd(
        nc, in_maps, core_ids=list(range(N_CORES)), trace=False
    )
    return np.concatenate(
        [res.results[i]["out"].reshape(BL, C) for i in range(N_CORES)],
        axis=0,
    )


# revision 4
# speedup vs baseline: 1.2507x; 1.0256x over previous
"""Dense MoE forward for Trainium2: 8-core data-parallel SPMD Bass/Tile kernel.

Reference computation (per token row x[b, :], all experts dense):
    gates = softmax(x @ Wg + bg)                      # [B, E]
    h_e   = relu(x @ W1[e] + b1[e])                   # [B, H] per expert
    eo_e  = h_e @ W2[e] + b2[e]                       # [B, C]
    out   = sum_e gates[:, e] * eo_e                  # [B, C]

Strategy (per core, B_loc = B/8 = 8192 tokens, data-parallel over cores):
  - x is cast to fp16 and pre-transposed on host to [128 d-part, macro,
    k-chunk, token] so the device does plain (non-transposing) DMA loads.
  - Tokens are processed in 1024-token super-macros (2 x 512-token halves).
    Layer 1 is feature-major: hT[e] = W1[e].T @ xT with fp32 PSUM
    accumulation over the 4 k-chunks of D=512.
  - Each W1 stationary [128d x 128h] is loaded once and feeds BOTH
    512-token halves (two matmuls into two PSUM banks). The tile
    legalizer emits one LDWEIGHTS per matmul unconditionally; hardware
    keeps the stationary across matmuls, so dedupe_ldweights() removes
    back-to-back duplicate InstLdweights from the final scheduled BIR
    (measured ~100ns each on HW). This is safe by construction: a
    duplicate is only removed when it is immediately adjacent to an
    identical load in the final PE program order and carries no
    semaphore waits/updates.
  - Gate logits token-major [tok, E]; softmax over the free dim (exp on
    ACT with multiplicative exp(bg), sum/reciprocal/normalize on DVE).
  - Layer 2 token-major: eo[t*128:, :] = hT-slice.T @ W2[e] accumulated
    in PSUM over the two h-chunks; gate-weighted combine and the b2 term
    on DVE via scalar_tensor_tensor with the per-partition gate column.
  - One merged output DMA per super-macro through a [B/128, 128, C] view
    of the output tensor.
"""

import contextlib

import numpy as np

from concourse import bacc, bass, mybir, tile
from concourse.bass_utils import run_bass_kernel_spmd

B, D, C, E, H = 65536, 512, 101, 8, 256
N_CORES = 8
BL = B // N_CORES     # 8192 tokens per core
MACRO = 512           # tokens per sub-macro (matmul moving dim)
NSUB = 4              # 128-token subtiles per sub-macro
SUB = 128
SMAC = 2              # sub-macros per super-macro (W1 stationary reuse)
SUPER = SMAC * MACRO  # 1024
DK = D // 128
HJ = H // 128

F16 = mybir.dt.float16
F32 = mybir.dt.float32
AF = mybir.ActivationFunctionType
ALU = mybir.AluOpType


def dedupe_ldweights(nc):
    """Remove back-to-back duplicate InstLdweights from the compiled BIR.

    Only an InstLdweights whose lowered access pattern is identical to the
    immediately preceding InstLdweights on the PE stream (no other weight
    load or transpose between), and which carries no semaphore waits or
    updates, is dropped. Matmuls between the two loads do not touch the
    PE weight registers, so the second load is redundant by definition.
    """
    removed = 0
    for blk in nc.m.functions[0].blocks:
        insts = blk.instructions
        new = []
        last_key = None
        changed = False
        for i in insts:
            tn = type(i).__name__
            if tn == "InstLdweights":
                key = (repr(i.ins[0]), str(i.perf_mode), str(i.tile_position),
                       str(i.is_transpose))
                if (key == last_key and not i.has_wait()
                        and not i.has_update()):
                    removed += 1
                    changed = True
                    continue
                last_key = key
            elif tn in ("InstMatmult", "InstMatmultMx"):
                if getattr(i, "is_transpose", None):
                    last_key = None
            new.append(i)
        if changed:
            blk.instructions = new
    return removed


def build_nc(n_tokens: int = BL, repeat: int = 1) -> bass.Bass:
    """repeat > 1 re-runs the whole body in a hardware loop (same data,
    same outputs) — used only for timing measurements via wall-clock slope."""
    assert n_tokens % SUPER == 0
    n_super = n_tokens // SUPER
    nm512 = n_tokens // MACRO

    nc = bacc.Bacc("TRN2", debug=False)

    xd = nc.dram_tensor("x", [128, nm512, DK * MACRO], F16,
                        kind="ExternalInput").ap()
    w1 = nc.dram_tensor("w1", [128, DK, E, H], F16, kind="ExternalInput").ap()
    w2 = nc.dram_tensor("w2", [128, HJ, E, C], F16, kind="ExternalInput").ap()
    wg = nc.dram_tensor("wg", [128, DK, E], F16, kind="ExternalInput").ap()
    b1 = nc.dram_tensor("b1", [128, HJ, E], F32, kind="ExternalInput").ap()
    b2 = nc.dram_tensor("b2", [128, E, C], F32, kind="ExternalInput").ap()
    bg = nc.dram_tensor("bg", [128, SMAC, NSUB, E], F32,
                        kind="ExternalInput").ap()
    # out viewed [a, 128, C]: token a*128+p — same bytes as [n_tokens, C]
    out = nc.dram_tensor("out", [n_tokens // SUB, SUB, C], F32,
                         kind="ExternalOutput").ap()

    with tile.TileContext(nc) as tc:
        with (
            tc.tile_pool(name="wpool", bufs=1) as wpool,
            tc.tile_pool(name="xpool", bufs=2) as xpool,
            tc.tile_pool(name="hpool", bufs=2) as hpool,
            tc.tile_pool(name="gpool", bufs=2) as gpool,
            tc.tile_pool(name="apool", bufs=2) as apool,
            tc.tile_pool(name="pgp", bufs=1, space="PSUM") as pgp,
            tc.tile_pool(name="php", bufs=4, space="PSUM") as php,
            tc.tile_pool(name="peop", bufs=3, space="PSUM") as peop,
        ):
            w1s = wpool.tile([128, DK, E, H], F16)
            w2s = wpool.tile([128, HJ, E, C], F16)
            wgs = wpool.tile([128, DK, E], F16)
            b1s = wpool.tile([128, HJ, E], F32)
            b2s = wpool.tile([128, E, C], F32)
            bgs = wpool.tile([128, SMAC, NSUB, E], F32)

            nc.sync.dma_start(out=w1s[:], in_=w1)
            nc.sync.dma_start(out=w2s[:], in_=w2)
            nc.sync.dma_start(out=wgs[:], in_=wg)
            nc.sync.dma_start(out=b1s[:], in_=b1)
            nc.sync.dma_start(out=b2s[:], in_=b2)
            nc.sync.dma_start(out=bgs[:], in_=bg)

            rep_ctx = (
                tc.For_i(0, repeat, 1) if repeat > 1
                else contextlib.nullcontext()
            )
            with rep_ctx:
                for M in range(n_super):
                    emit_super(nc, M, xd, out, w1s, w2s, wgs, b1s, b2s, bgs,
                               xpool, hpool, gpool, apool, pgp, php, peop)

    nc.compile()
    dedupe_ldweights(nc)
    return nc


def emit_super(nc, M, xd, out, w1s, w2s, wgs, b1s, b2s, bgs,
               xpool, hpool, gpool, apool, pgp, php, peop):
    xt = xpool.tile([128, SMAC, DK, MACRO], F16, tag="xt")
    nc.sync.dma_start(out=xt[:], in_=xd[:, SMAC * M:SMAC * (M + 1)])

    # ---- gates (token-major logits, softmax over E) ----
    # u = exp(x @ Wg) * exp(bg); normalize by its row sum on DVE.
    # Emitted lazily after L1(e=0): the gate matmuls then run as one burst
    # between experts instead of being scheduler-woven between the L1
    # same-stationary quads (which would break LDWEIGHTS dedupe adjacency).
    def emit_gates():
        pg = pgp.tile([128, SMAC, NSUB, E], F32, tag="pg")
        for s in range(SMAC):
            for t in range(NSUB):
                for k in range(DK):
                    nc.tensor.matmul(
                        pg[:, s, t],
                        lhsT=xt[:, s, k, t * SUB:(t + 1) * SUB],
                        rhs=wgs[:, k],
                        start=(k == 0),
                        stop=(k == DK - 1),
                    )
        u = gpool.tile([128, SMAC, NSUB, E], F32, tag="u")
        nc.scalar.activation(u[:], pg[:], AF.Exp)
        u2 = gpool.tile([128, SMAC, NSUB, E], F32, tag="u2")
        nc.vector.tensor_mul(u2[:], u[:], bgs[:])
        sm = gpool.tile([128, SMAC, NSUB], F32, tag="s")
        nc.vector.reduce_sum(sm[:], u2[:], axis=mybir.AxisListType.X)
        r = gpool.tile([128, SMAC, NSUB], F32, tag="r")
        nc.vector.reciprocal(r[:], sm[:])
        g = gpool.tile([128, SMAC, NSUB, E], F32, tag="g")
        for s in range(SMAC):
            for t in range(NSUB):
                nc.vector.tensor_scalar_mul(g[:, s, t], u2[:, s, t],
                                            r[:, s, t:t + 1])
        return g

    acc = apool.tile([128, SMAC * NSUB, C], F32, tag="acc")

    def emit_l2(e, ht, g):
        """Layer 2 + gated combine for expert e (token-major).
        acc += g_e * (h_e @ W2_e) + g_e * b2_e; the b2 term uses the
        partition-replicated b2s tile on DVE (no PE matmul)."""
        for t in range(SMAC * NSUB):
            peo = peop.tile([128, C], F32, tag="peo", name="peo")
            for j in range(HJ):
                nc.tensor.matmul(
                    peo[:],
                    lhsT=ht[:, j, t * SUB:(t + 1) * SUB],
                    rhs=w2s[:, j, e],
                    start=(j == 0),
                    stop=(j == HJ - 1),
                )
            gcol = g[:, t // NSUB, t % NSUB, e:e + 1]
            if e == 0:
                nc.vector.tensor_scalar_mul(acc[:, t], peo[:], gcol)
            else:
                nc.vector.scalar_tensor_tensor(
                    acc[:, t], peo[:], gcol, acc[:, t],
                    op0=ALU.mult, op1=ALU.add,
                )
            nc.vector.scalar_tensor_tensor(
                acc[:, t], b2s[:, e], gcol, acc[:, t],
                op0=ALU.mult, op1=ALU.add,
            )

    # ---- experts: each W1 stationary feeds both 512-token halves; L1(e)
    # is emitted before L2(e-1) so the PE always has independent matmul
    # work while ACT runs relu(e). ----
    pending = None
    g = None
    for e in range(E):
        ht = hpool.tile([128, HJ, SUPER], F16, tag="ht", name="ht")
        for j in range(HJ):
            phs = [
                php.tile([128, MACRO], F32, tag="ph", name="ph")
                for _ in range(SMAC)
            ]
            for k in range(DK):
                for h in range(SMAC):
                    nc.tensor.matmul(
                        phs[h][:],
                        lhsT=w1s[:, k, e, j * 128:(j + 1) * 128],
                        rhs=xt[:, h, k],
                        start=(k == 0),
                        stop=(k == DK - 1),
                    )
            for h in range(SMAC):
                nc.scalar.activation(
                    ht[:, j, h * MACRO:(h + 1) * MACRO], phs[h][:],
                    AF.Relu, bias=b1s[:, j, e:e + 1]
                )
        if e == 0:
            g = emit_gates()
        if pending is not None:
            emit_l2(*pending, g)
        pending = (e, ht)
    emit_l2(*pending, g)

    # ---- store: one DMA for the whole super-macro ----
    a0 = M * (SUPER // SUB)
    nc.sync.dma_start(
        out=out[a0:a0 + SMAC * NSUB].rearrange("a p c -> p a c"),
        in_=acc[:],
    )


def _prep_weights(W1, b1, W2, b2, Wg, bg):
    w1p = np.ascontiguousarray(
        W1.astype(np.float16).transpose(1, 0, 2).reshape(DK, 128, E, H)
        .transpose(1, 0, 2, 3)
    )
    w2p = np.ascontiguousarray(
        W2.astype(np.float16).transpose(1, 0, 2).reshape(HJ, 128, E, C)
        .transpose(1, 0, 2, 3)
    )
    wgp = np.ascontiguousarray(
        Wg.astype(np.float16).reshape(DK, 128, E).transpose(1, 0, 2)
    )
    b1p = np.ascontiguousarray(
        b1.astype(np.float32).T.reshape(HJ, 128, E).transpose(1, 0, 2)
    )
    b2p = np.ascontiguousarray(
        np.broadcast_to(b2.astype(np.float32), (128, E, C))
    )
    bgp = np.ascontiguousarray(np.broadcast_to(
        np.exp(bg).astype(np.float32), (128, SMAC, NSUB, E)
    ))
    return w1p, w2p, wgp, b1p, b2p, bgp


def prep_x(x16_core):
    """[BL, 512] fp16 -> [128, n_macro512, DK*512], host-transposed so
    element [p, m, k*512+t] = x[m*512+t, k*128+p]."""
    nm = x16_core.shape[0] // MACRO
    return np.ascontiguousarray(
        x16_core.reshape(nm, MACRO, DK, 128).transpose(3, 0, 2, 1)
        .reshape(128, nm, DK * MACRO)
    )


def make_in_maps(inputs):
    x16 = np.asarray(inputs["x"], np.float32).astype(np.float16)
    w1p, w2p, wgp, b1p, b2p, bgp = _prep_weights(
        np.asarray(inputs["W1"], np.float32),
        np.asarray(inputs["b1"], np.float32),
        np.asarray(inputs["W2"], np.float32),
        np.asarray(inputs["b2"], np.float32),
        np.asarray(inputs["Wg"], np.float32),
        np.asarray(inputs["bg"], np.float32))
    return [
        {
            "x": prep_x(x16[i * BL:(i + 1) * BL]),
            "w1": w1p, "w2": w2p, "wg": wgp,
            "b1": b1p, "b2": b2p, "bg": bgp,
        }
        for i in range(N_CORES)
    ]


_CACHE: dict = {}


def kernel(x, W1, b1, W2, b2, Wg, bg):
    if "nc" not in _CACHE:
        _CACHE["nc"] = build_nc()
    nc = _CACHE["nc"]

    in_maps = make_in_maps({
        "x": x, "W1": W1, "b1": b1, "W2": W2, "b2": b2, "Wg": Wg, "bg": bg,
    })
    res = run_bass_kernel_spmd(
        nc, in_maps, core_ids=list(range(N_CORES)), trace=False
    )
    return np.concatenate(
        [res.results[i]["out"].reshape(BL, C) for i in range(N_CORES)],
        axis=0,
    )
